# revision 29
# baseline (speedup 1.0000x reference)
"""Trainium2 Bass kernel for DemoHybridAttention (sliding-window + memory-token
attention block: B=2 S=2048 D=512 H=16 hd=32 window=+/-128, 16 mem tokens).

Sharding: 8 cores = 2 batches x 4 query-blocks of 512 tokens. Each core
computes its query block end-to-end (all 16 heads); no collectives; the host
concatenates the per-core [512, 512] outputs.

Device-side layout: scores are computed transposed ([keys, queries]) so the
softmaxed weights feed the PV matmul as the stationary operand with no
transposes. Softmax skips the row-max subtraction (scores are O(1) Gaussian;
exp is safe in fp32). PV emits ctx in [query, head*hd] layout with a
ones-column producing the softmax sums per query partition, so normalization
is one reciprocal + one broadcast multiply per query tile; the normalized ctx
is transposed on the PE for the output projection. Matmuls are bf16 with fp32
PSUM accumulation; the band mask is one merged multiplicative 0/1 template per
head; halo keys off the sequence edge die via a per-partition -1e30 exp bias.
"""

import os
import sys
from contextlib import ExitStack

import numpy as np
import ml_dtypes

# concourse (Bass/Tile) normally arrives on sys.path via sitecustomize; be
# defensive in case the grading harness runs with a bare interpreter.
for _p in ("/opt/trn_rl_repo", "/root/.axon_site/_ro/trn_rl_repo"):
    if os.path.isdir(_p) and _p not in sys.path:
        sys.path.append(_p)

B = 2
S = 2048
D = 512
H = 16
HD = 32
W2 = 128          # window // 2
MEM = 16
NEG = -1.0e30
SCALE = HD ** -0.5

NCORES = 8
SB = 512          # queries per core
HALO = W2         # key halo on each side
KC = SB + 2 * HALO        # content keys per core (768)
NKC = KC // 128           # content key chunks (6)
KT = KC + MEM             # total keys per core (784, mem tokens first)
VS = HD + 1               # v interleave stride (32 v dims + ones col)
# content chunks packed tightly into the weight tile / mask template:
# groups (0,1), (2,), (3,), (4,5) each span 384 cols and share one exp
CBASE = [0, 128, 384, 768, 1152, 1408]
CGROUPS = [(0, 1), (2,), (3,), (4, 5)]
WCW = 1536

BF16 = ml_dtypes.bfloat16

_CACHE = {}
LAST_RESULT = None


def _chunk_qwin(c):
    """Query window [qw0, qw1) that content-key chunk c (128 keys) attends to."""
    qw0 = max(0, 128 * c - 2 * W2)
    qw1 = min(SB, 128 * c + 128)
    return qw0, qw1


def _build(has_bias):
    import concourse.tile as tile
    from concourse import bacc, mybir

    f32 = mybir.dt.float32
    bf16 = mybir.dt.bfloat16
    EXP = mybir.ActivationFunctionType.Exp

    nc = bacc.Bacc("TRN2", target_bir_lowering=False, debug=False,
                   num_devices=NCORES)

    xq_d = nc.dram_tensor("xq", [D, SB], bf16, kind="ExternalInput").ap()
    xk_d = nc.dram_tensor("xk", [D, KT], bf16, kind="ExternalInput").ap()
    xv_d = nc.dram_tensor("xv", [D, KT], bf16, kind="ExternalInput").ap()
    wq_d = nc.dram_tensor("wq", [D, D], bf16, kind="ExternalInput").ap()
    wk_d = nc.dram_tensor("wk", [D, D], bf16, kind="ExternalInput").ap()
    wv_d = nc.dram_tensor("wv", [D, D], bf16, kind="ExternalInput").ap()
    wo_d = nc.dram_tensor("wo", [D, D], bf16, kind="ExternalInput").ap()
    # aux: [:, 0:WCW] merged mask template (with key-validity folded in),
    # [:, WCW:WCW+128] identity
    AUXW = WCW + 128
    aux_d = nc.dram_tensor("aux", [128, AUXW], bf16, kind="ExternalInput").ap()
    if has_bias:
        bq_d = nc.dram_tensor("bq", [1, D], bf16, kind="ExternalInput").ap()
        bk_d = nc.dram_tensor("bk", [1, D], bf16, kind="ExternalInput").ap()
        bv_d = nc.dram_tensor("bv", [1, D], bf16, kind="ExternalInput").ap()
        bo_d = nc.dram_tensor("bo", [1, D], bf16, kind="ExternalInput").ap()
    out_d = nc.dram_tensor("out", [SB, D], f32, kind="ExternalOutput").ap()

    with tile.TileContext(nc) as tc, ExitStack() as ctx:
        const = ctx.enter_context(tc.tile_pool(name="const", bufs=1))
        psum = ctx.enter_context(tc.tile_pool(name="psum", bufs=2, space="PSUM"))
        work = ctx.enter_context(tc.tile_pool(name="work", bufs=2))

        # ---- load everything ----
        xq = const.tile([128, 4 * SB], bf16)
        xk = const.tile([128, 4 * KT], bf16)
        xv = const.tile([128, 4 * KT], bf16)
        wq = const.tile([128, 4 * D], bf16)
        wk = const.tile([128, 4 * D], bf16)
        wv = const.tile([128, 4 * D], bf16)
        wo = const.tile([128, 4 * D], bf16)
        # first projection's operands land first so PE starts ASAP
        for sb_t, dr in ((wq, wq_d), (xq, xq_d), (wk, wk_d), (xk, xk_d),
                         (wv, wv_d), (xv, xv_d), (wo, wo_d)):
            n = dr.shape[1]
            nc.sync.dma_start(
                sb_t[:, 0:4 * n].rearrange("p (c n) -> p c n", c=4),
                dr.rearrange("(c p) n -> p c n", p=128))
        aux = const.tile([128, AUXW], bf16)
        nc.sync.dma_start(aux[:], aux_d[:])
        maskC = aux[:, 0:WCW]
        ident = aux[:, WCW:WCW + 128]
        if has_bias:
            bq = const.tile([1, D], bf16)
            bk = const.tile([1, D], bf16)
            bv = const.tile([1, D], bf16)
            bo = const.tile([1, D], bf16)
            nc.sync.dma_start(bq[:], bq_d[:])
            nc.sync.dma_start(bk[:], bk_d[:])
            nc.sync.dma_start(bv[:], bv_d[:])
            nc.sync.dma_start(bo[:], bo_d[:])
            ones_row = const.tile([1, KT], bf16)
            nc.vector.memset(ones_row[:], 1.0)

        # ---- projections ----
        # qT[hg]: [128 (4 heads x 32 hd), 512 q]
        qT = const.tile([128, 4 * SB], bf16)
        kT = const.tile([128, 4 * KT], bf16)
        vt = const.tile([128, NKC * H * VS], bf16)
        vt_mem4 = const.tile([128, H * VS], bf16)

        def emit_qT(hg):
            ps = psum.tile([128, SB], f32, tag="proj", name=f"psq{hg}")
            for kc in range(4):
                nc.tensor.matmul(
                    ps[:],
                    wq[:, D * kc + 128 * hg:D * kc + 128 * (hg + 1)],
                    xq[:, SB * kc:SB * (kc + 1)],
                    start=(kc == 0), stop=(kc == 3 and not has_bias),
                )
            if has_bias:
                nc.tensor.matmul(ps[:], bq[:, 128 * hg:128 * (hg + 1)],
                                 ones_row[:, 0:SB], start=False, stop=True)
            nc.vector.tensor_copy(qT[:, SB * hg:SB * (hg + 1)], ps[:])

        def emit_kT(hg):
            # [128 (4 heads x 32 hd), 784 keys] (mem tokens first)
            for (o0, o1) in ((0, 512), (512, KT)):
                ps = psum.tile([128, 512], f32, tag="proj", name=f"psk{hg}_{o0}")
                for kc in range(4):
                    nc.tensor.matmul(
                        ps[:, 0:o1 - o0],
                        wk[:, D * kc + 128 * hg:D * kc + 128 * (hg + 1)],
                        xk[:, KT * kc + o0:KT * kc + o1],
                        start=(kc == 0), stop=(kc == 3 and not has_bias),
                    )
                if has_bias:
                    nc.tensor.matmul(ps[:, 0:o1 - o0],
                                     bk[:, 128 * hg:128 * (hg + 1)],
                                     ones_row[:, o0:o1], start=False, stop=True)
                nc.vector.tensor_copy(kT[:, KT * hg + o0:KT * hg + o1],
                                      ps[:, 0:o1 - o0])

        def emit_v(t):
            # v interleaved per head with a ones column. content chunks go to
            # vt [128 keys, 6*16*33]; the 16 mem rows are replicated at
            # partition bases 0/32/64/96 in vt_mem4.
            rows, tok0 = (MEM, 0) if t == 0 else (128, MEM + 128 * (t - 1))
            ps = psum.tile([128, D], f32, tag="proj", name=f"psv{t}")
            for kc in range(4):
                nc.tensor.matmul(
                    ps[0:rows, :],
                    xv[:, KT * kc + tok0:KT * kc + tok0 + rows],
                    wv[:, D * kc:D * (kc + 1)],
                    start=(kc == 0), stop=(kc == 3 and not has_bias),
                )
            if has_bias:
                nc.tensor.matmul(ps[0:rows, :], ones_row[:, tok0:tok0 + rows],
                                 bv[:], start=False, stop=True)
            if t == 0:
                for b in range(4):
                    dst = vt_mem4[32 * b:32 * b + MEM, :]
                    nc.scalar.copy(
                        dst.rearrange("p (h v) -> p h v", h=H)[:, :, 0:HD],
                        ps[0:MEM, :].rearrange("p (h v) -> p h v", h=H))
                    nc.vector.memset(
                        dst.rearrange("p (h v) -> p h v", h=H)[:, :, HD:HD + 1],
                        1.0)
            else:
                dst = vt[:, (t - 1) * H * VS:t * H * VS]
                nc.vector.tensor_copy(
                    dst.rearrange("p (h v) -> p h v", h=H)[:, :, 0:HD],
                    ps[:].rearrange("p (h v) -> p h v", h=H))
                nc.vector.memset(
                    dst.rearrange("p (h v) -> p h v", h=H)[:, :, HD:HD + 1], 1.0)

        # interleave so heads 0-7 (half 0) can start attention early
        for hg in range(4):
            emit_qT(hg)
        for hg in range(4):
            emit_kT(hg)
        for t in range(7):
            emit_v(t)

        # ---- attention: 4 rounds of one head-group (4 heads) each ----
        # ctx psum per round: [128, 3*132] (qs 0-2) + [128, 132] (qs 3), so
        # all four query tiles fit two banks and the score pool can go deep.
        stg = const.tile([128, 4 * H * VS], f32)   # qs-major, head h at 33h
        ctxn = const.tile([128, 4 * SB], bf16)     # [q, D] normalized, per qs
        ctxT = const.tile([128, 4 * SB], bf16)
        STG = H * VS                               # 528 cols per qs block
        for hg in range(4):
            # mem-key scores, 4 heads stacked in one psum
            ps_m = psum.tile([128, SB], f32, tag="proj")
            for hh in range(4):
                nc.tensor.matmul(
                    ps_m[32 * hh:32 * hh + MEM, :],
                    kT[32 * hh:32 * (hh + 1), KT * hg:KT * hg + MEM],
                    qT[32 * hh:32 * (hh + 1), SB * hg:SB * (hg + 1)],
                    start=True, stop=True, tile_position=(32 * hh, 32 * hh),
                )
            wm4 = work.tile([128, SB], bf16, tag="wm", bufs=2)
            nc.scalar.activation(wm4[:], ps_m[:], EXP)

            ctx_a = psum.tile([128, 3 * 4 * VS], f32, tag="ctx", bufs=2,
                              name=f"ctxa{hg}")
            ctx_b = psum.tile([128, 4 * VS], f32, tag="ctx", bufs=2,
                              name=f"ctxb{hg}")
            for hh in range(4):
                h = 4 * hg + hh
                kTh = kT[32 * hh:32 * (hh + 1), KT * hg:KT * (hg + 1)]
                qTh = qT[32 * hh:32 * (hh + 1), SB * hg:SB * (hg + 1)]

                wc = work.tile([128, WCW], bf16, tag="wc", bufs=4)
                for grp in CGROUPS:
                    gbase = CBASE[grp[0]]
                    ps_s = psum.tile([128, 512], f32, tag="sc", bufs=4)
                    gspan = 0
                    for c in grp:
                        qw0, qw1 = _chunk_qwin(c)
                        lc = CBASE[c] - gbase
                        nc.tensor.matmul(
                            ps_s[:, lc:lc + qw1 - qw0],
                            kTh[:, MEM + 128 * c:MEM + 128 * (c + 1)],
                            qTh[:, qw0:qw1], start=True, stop=True,
                            tile_position=(32 * hh, 0),
                        )
                        gspan = lc + qw1 - qw0
                    nc.scalar.activation(wc[:, gbase:gbase + gspan],
                                         ps_s[:, 0:gspan], EXP)
                nc.vector.tensor_mul(wc[:], wc[:], maskC[:])

                for qs in range(4):
                    if qs < 3:
                        out_ps = ctx_a[:, 4 * VS * qs + VS * hh:
                                       4 * VS * qs + VS * (hh + 1)]
                    else:
                        out_ps = ctx_b[:, VS * hh:VS * (hh + 1)]
                    nc.tensor.matmul(
                        out_ps,
                        wm4[32 * hh:32 * hh + MEM, 128 * qs:128 * (qs + 1)],
                        vt_mem4[32 * hh:32 * hh + MEM, VS * h:VS * (h + 1)],
                        start=True, stop=False, skip_group_check=True,
                        tile_position=(32 * hh, 0),
                    )
                    for ci, c in enumerate((qs, qs + 1, qs + 2)):
                        qw0, _ = _chunk_qwin(c)
                        nc.tensor.matmul(
                            out_ps,
                            wc[:, CBASE[c] + 128 * qs - qw0:
                                CBASE[c] + 128 * qs - qw0 + 128],
                            vt[:, c * H * VS + VS * h:c * H * VS + VS * (h + 1)],
                            start=False, stop=(ci == 2), skip_group_check=True,
                        )
            # evac this round's ctx into stg (head h at col 528*qs + 33*h)
            nc.scalar.copy(
                stg[:].rearrange("p (q n) -> p q n", q=4)[
                    :, 0:3, 4 * VS * hg:4 * VS * (hg + 1)],
                ctx_a[:].rearrange("p (q n) -> p q n", q=3))
            nc.scalar.copy(stg[:, 3 * STG + 4 * VS * hg:3 * STG + 4 * VS * (hg + 1)],
                           ctx_b[:])
            # normalize + transpose this round's heads (D-block dc == hg)
            for qs in range(4):
                blk = stg[:, STG * qs + 4 * VS * hg:
                          STG * qs + 4 * VS * (hg + 1)].rearrange(
                    "p (h v) -> p h v", h=4)
                rec = work.tile([128, 4], f32, tag="rec")
                nc.vector.reciprocal(rec[:].unsqueeze(2), blk[:, :, HD:HD + 1])
                nc.vector.tensor_mul(
                    ctxn[:, SB * qs + 128 * hg:SB * qs + 128 * (hg + 1)]
                    .rearrange("p (h v) -> p h v", h=4),
                    blk[:, :, 0:HD],
                    rec[:].unsqueeze(2).broadcast_to([128, 4, HD]))
                ps_t = psum.tile([128, 128], bf16, tag="sc", bufs=4)
                nc.tensor.transpose(
                    ps_t[:],
                    ctxn[:, SB * qs + 128 * hg:SB * qs + 128 * (hg + 1)],
                    ident)
                nc.vector.tensor_copy(
                    ctxT[:, SB * hg + 128 * qs:SB * hg + 128 * (qs + 1)],
                    ps_t[:])

        # ---- output projection ----
        o_sb = const.tile([128, 4 * D], f32)
        for t in range(4):
            ps_o = psum.tile([128, D], f32, tag="proj")
            if has_bias:
                nc.tensor.matmul(ps_o[:], ones_row[:, 0:128], bo[:],
                                 start=True, stop=False)
            for dc in range(4):
                nc.tensor.matmul(
                    ps_o[:],
                    ctxT[:, SB * dc + 128 * t:SB * dc + 128 * (t + 1)],
                    wo[:, D * dc:D * (dc + 1)],
                    start=(dc == 0 and not has_bias), stop=(dc == 3),
                )
            nc.vector.tensor_copy(o_sb[:, D * t:D * (t + 1)], ps_o[:])
        nc.sync.dma_start(
            out_d.rearrange("(t p) n -> p t n", p=128),
            o_sb[:].rearrange("p (t n) -> p t n", t=4))

    nc.compile()
    return nc


def _host_prep(inputs, has_bias):
    """Build the 8 per-core input maps."""
    q = np.asarray(inputs["query"], np.float32)
    k = np.asarray(inputs["key"], np.float32)
    v = np.asarray(inputs["value"], np.float32)
    mem = np.asarray(inputs["mem"], np.float32)[0]          # [16, 512]
    Wq = np.asarray(inputs["Wq"], np.float32) * SCALE
    bq = np.asarray(inputs["bq"], np.float32) * SCALE

    shared = {
        "wq": Wq.astype(BF16),
        "wk": np.asarray(inputs["Wk"], np.float32).astype(BF16),
        "wv": np.asarray(inputs["Wv"], np.float32).astype(BF16),
        "wo": np.asarray(inputs["Wo"], np.float32).astype(BF16),
    }
    if has_bias:
        shared["bq"] = bq.reshape(1, D).astype(BF16)
        for nm in ("bk", "bv", "bo"):
            shared[nm] = np.asarray(inputs[nm], np.float32).reshape(1, D).astype(BF16)

    memT = mem.T.astype(BF16)                                # [512, 16]
    in_maps = []
    for c in range(NCORES):
        b, blk = divmod(c, 4)
        s0 = blk * SB
        g0, g1 = s0 - HALO, s0 + SB + HALO
        lo, hi = max(0, g0), min(S, g1)
        xk_h = np.zeros((KC, D), np.float32)
        xv_h = np.zeros((KC, D), np.float32)
        xk_h[lo - g0:hi - g0] = k[b, lo:hi]
        xv_h[lo - g0:hi - g0] = v[b, lo:hi]
        # per-core mask template: 0/1 band per chunk, tightly packed per
        # CBASE, with out-of-sequence halo keys (whole partitions) zeroed
        cc = np.arange(KC) + g0
        kvalid = ((cc >= 0) & (cc < S)).astype(np.float32)
        maskC = np.zeros((128, WCW), np.float32)
        for ch in range(NKC):
            qw0, qw1 = _chunk_qwin(ch)
            p = np.arange(128)[:, None]
            n = np.arange(qw1 - qw0)[None, :]
            d = p + (128 * ch - qw0) - n
            band = ((d >= 0) & (d <= 2 * W2)).astype(np.float32)
            band *= kvalid[128 * ch:128 * (ch + 1)][:, None]
            maskC[:, CBASE[ch]:CBASE[ch] + qw1 - qw0] = band
        aux = np.zeros((128, WCW + 128), np.float32)
        aux[:, 0:WCW] = maskC
        aux[:, WCW:] = np.eye(128, dtype=np.float32)
        in_maps.append({
            "xq": np.ascontiguousarray(q[b, s0:s0 + SB].T).astype(BF16),
            "xk": np.concatenate([memT, xk_h.T.astype(BF16)], axis=1),
            "xv": np.concatenate([memT, xv_h.T.astype(BF16)], axis=1),
            "aux": aux.astype(BF16),
            **shared,
        })
    return in_maps


def kernel(**inputs):
    global LAST_RESULT
    from concourse.bass_utils import run_bass_kernel_spmd

    has_bias = any(
        np.any(np.asarray(inputs[n]) != 0) for n in ("bq", "bk", "bv", "bo"))
    key = ("nc", has_bias)
    if key not in _CACHE:
        _CACHE[key] = _build(has_bias)
    nc = _CACHE[key]

    in_maps = _host_prep(inputs, has_bias)
    res = run_bass_kernel_spmd(nc, in_maps, list(range(NCORES)))
    LAST_RESULT = res

    out = np.empty((B, S, D), np.float32)
    for c in range(NCORES):
        b, blk = divmod(c, 4)
        out[b, blk * SB:(blk + 1) * SB] = res.results[c]["out"]
    return out


# revision 36
# speedup vs baseline: 1.2051x; 1.2051x over previous
"""Trainium2 Bass kernel for DemoHybridAttention (sliding-window + memory-token
attention block: B=2 S=2048 D=512 H=16 hd=32 window=+/-128, 16 mem tokens).

Sharding: 8 cores = 2 batches x 4 query-blocks of 512 tokens. Each core
computes its query block end-to-end (all 16 heads); no collectives; the host
concatenates the per-core [512, 512] outputs.

Device-side layout: scores are computed transposed ([keys, queries]) so the
softmaxed weights feed the PV matmul as the stationary operand with no
transposes. Softmax skips the row-max subtraction (scores are O(1) Gaussian;
exp is safe in fp32). PV emits ctx in [query, head*hd] layout with a
ones-column producing the softmax sums per query partition, so normalization
is one reciprocal + one broadcast multiply per query tile; the normalized ctx
is transposed on the PE for the output projection. Matmuls are bf16 with fp32
PSUM accumulation; the band mask is one merged multiplicative 0/1 template per
head; halo keys off the sequence edge die via a per-partition -1e30 exp bias.
"""

import os
import sys
from contextlib import ExitStack

import numpy as np
import ml_dtypes

# concourse (Bass/Tile) normally arrives on sys.path via sitecustomize; be
# defensive in case the grading harness runs with a bare interpreter.
for _p in ("/opt/trn_rl_repo", "/root/.axon_site/_ro/trn_rl_repo"):
    if os.path.isdir(_p) and _p not in sys.path:
        sys.path.append(_p)

B = 2
S = 2048
D = 512
H = 16
HD = 32
W2 = 128          # window // 2
MEM = 16
NEG = -1.0e30
SCALE = HD ** -0.5

NCORES = 8
SB = 512          # queries per core
HALO = W2         # key halo on each side
KC = SB + 2 * HALO        # content keys per core (768)
NKC = KC // 128           # content key chunks (6)
KT = KC + MEM             # total keys per core (784, mem tokens first)
VS = HD + 1               # v interleave stride (32 v dims + ones col)
# content chunks packed tightly into the weight tile / mask template:
# groups (0,1), (2,), (3,), (4,5) each span 384 cols and share one exp
CBASE = [0, 128, 384, 768, 1152, 1408]
CGROUPS = [(0, 1), (2,), (3,), (4, 5)]
WCW = 1536

BF16 = ml_dtypes.bfloat16

_CACHE = {}
LAST_RESULT = None


def _chunk_qwin(c):
    """Query window [qw0, qw1) that content-key chunk c (128 keys) attends to."""
    qw0 = max(0, 128 * c - 2 * W2)
    qw1 = min(SB, 128 * c + 128)
    return qw0, qw1


def _build(has_bias):
    import concourse.tile as tile
    from concourse import bacc, mybir

    f32 = mybir.dt.float32
    bf16 = mybir.dt.bfloat16
    EXP = mybir.ActivationFunctionType.Exp

    nc = bacc.Bacc("TRN2", target_bir_lowering=False, debug=False,
                   num_devices=NCORES)

    xq_d = nc.dram_tensor("xq", [D, SB], bf16, kind="ExternalInput").ap()
    xk_d = nc.dram_tensor("xk", [D, KT], bf16, kind="ExternalInput").ap()
    xv_d = nc.dram_tensor("xv", [D, KT], bf16, kind="ExternalInput").ap()
    wq_d = nc.dram_tensor("wq", [D, D], bf16, kind="ExternalInput").ap()
    wk_d = nc.dram_tensor("wk", [D, D], bf16, kind="ExternalInput").ap()
    wv_d = nc.dram_tensor("wv", [D, D], bf16, kind="ExternalInput").ap()
    wo_d = nc.dram_tensor("wo", [D, D], bf16, kind="ExternalInput").ap()
    # aux: [:, 0:WCW] merged mask template (with key-validity folded in),
    # [:, WCW:WCW+128] identity
    AUXW = WCW + 128
    aux_d = nc.dram_tensor("aux", [128, AUXW], bf16, kind="ExternalInput").ap()
    if has_bias:
        bq_d = nc.dram_tensor("bq", [1, D], bf16, kind="ExternalInput").ap()
        bk_d = nc.dram_tensor("bk", [1, D], bf16, kind="ExternalInput").ap()
        bv_d = nc.dram_tensor("bv", [1, D], bf16, kind="ExternalInput").ap()
        bo_d = nc.dram_tensor("bo", [1, D], bf16, kind="ExternalInput").ap()
    out_d = nc.dram_tensor("out", [SB, D], f32, kind="ExternalOutput").ap()

    with tile.TileContext(nc) as tc, ExitStack() as ctx:
        const = ctx.enter_context(tc.tile_pool(name="const", bufs=1))
        psum = ctx.enter_context(tc.tile_pool(name="psum", bufs=2, space="PSUM"))
        work = ctx.enter_context(tc.tile_pool(name="work", bufs=2))

        # ---- load everything ----
        xq = const.tile([128, 4 * SB], bf16)
        xk = const.tile([128, 4 * KT], bf16)
        xv = const.tile([128, 4 * KT], bf16)
        wq = const.tile([128, 4 * D], bf16)
        wk = const.tile([128, 4 * D], bf16)
        wv = const.tile([128, 4 * D], bf16)
        wo = const.tile([128, 4 * D], bf16)
        # first projection's operands land first so PE starts ASAP
        for sb_t, dr in ((wq, wq_d), (xq, xq_d), (wk, wk_d), (xk, xk_d),
                         (wv, wv_d), (xv, xv_d)):
            n = dr.shape[1]
            nc.sync.dma_start(
                sb_t[:, 0:4 * n].rearrange("p (c n) -> p c n", c=4),
                dr.rearrange("(c p) n -> p c n", p=128))
        aux = const.tile([128, AUXW], bf16)
        nc.sync.dma_start(aux[:], aux_d[:])
        maskC = aux[:, 0:WCW]
        ident = aux[:, WCW:WCW + 128]
        # wo is only needed by the output projection at the very end
        nc.sync.dma_start(
            wo[:].rearrange("p (c n) -> p c n", c=4),
            wo_d.rearrange("(c p) n -> p c n", p=128))
        if has_bias:
            bq = const.tile([1, D], bf16)
            bk = const.tile([1, D], bf16)
            bv = const.tile([1, D], bf16)
            bo = const.tile([1, D], bf16)
            nc.sync.dma_start(bq[:], bq_d[:])
            nc.sync.dma_start(bk[:], bk_d[:])
            nc.sync.dma_start(bv[:], bv_d[:])
            nc.sync.dma_start(bo[:], bo_d[:])
            ones_row = const.tile([1, KT], bf16)
            nc.vector.memset(ones_row[:], 1.0)

        # ---- projections ----
        # qT[hg]: [128 (4 heads x 32 hd), 512 q]
        qT = const.tile([128, 4 * SB], bf16)
        kT = const.tile([128, 4 * KT], bf16)
        vt = const.tile([128, NKC * H * VS], bf16)
        vt_mem4 = const.tile([128, H * VS], bf16)

        def emit_qT(hg):
            ps = psum.tile([128, SB], f32, tag="proj", name=f"psq{hg}")
            for kc in range(4):
                nc.tensor.matmul(
                    ps[:],
                    wq[:, D * kc + 128 * hg:D * kc + 128 * (hg + 1)],
                    xq[:, SB * kc:SB * (kc + 1)],
                    start=(kc == 0), stop=(kc == 3 and not has_bias),
                )
            if has_bias:
                nc.tensor.matmul(ps[:], bq[:, 128 * hg:128 * (hg + 1)],
                                 ones_row[:, 0:SB], start=False, stop=True)
            nc.vector.tensor_copy(qT[:, SB * hg:SB * (hg + 1)], ps[:])

        def emit_kT(hg):
            # [128 (4 heads x 32 hd), 784 keys] (mem tokens first)
            for (o0, o1) in ((0, 512), (512, KT)):
                ps = psum.tile([128, 512], f32, tag="proj", name=f"psk{hg}_{o0}")
                for kc in range(4):
                    nc.tensor.matmul(
                        ps[:, 0:o1 - o0],
                        wk[:, D * kc + 128 * hg:D * kc + 128 * (hg + 1)],
                        xk[:, KT * kc + o0:KT * kc + o1],
                        start=(kc == 0), stop=(kc == 3 and not has_bias),
                    )
                if has_bias:
                    nc.tensor.matmul(ps[:, 0:o1 - o0],
                                     bk[:, 128 * hg:128 * (hg + 1)],
                                     ones_row[:, o0:o1], start=False, stop=True)
                nc.vector.tensor_copy(kT[:, KT * hg + o0:KT * hg + o1],
                                      ps[:, 0:o1 - o0])

        def emit_v(t):
            # v interleaved per head with a ones column. content chunks go to
            # vt [128 keys, 6*16*33]; the 16 mem rows are replicated at
            # partition bases 0/32/64/96 in vt_mem4.
            rows, tok0 = (MEM, 0) if t == 0 else (128, MEM + 128 * (t - 1))
            ps = psum.tile([128, D], f32, tag="proj", name=f"psv{t}")
            for kc in range(4):
                nc.tensor.matmul(
                    ps[0:rows, :],
                    xv[:, KT * kc + tok0:KT * kc + tok0 + rows],
                    wv[:, D * kc:D * (kc + 1)],
                    start=(kc == 0), stop=(kc == 3 and not has_bias),
                )
            if has_bias:
                nc.tensor.matmul(ps[0:rows, :], ones_row[:, tok0:tok0 + rows],
                                 bv[:], start=False, stop=True)
            if t == 0:
                for b in range(4):
                    dst = vt_mem4[32 * b:32 * b + MEM, :]
                    nc.scalar.copy(
                        dst.rearrange("p (h v) -> p h v", h=H)[:, :, 0:HD],
                        ps[0:MEM, :].rearrange("p (h v) -> p h v", h=H))
                    nc.vector.memset(
                        dst.rearrange("p (h v) -> p h v", h=H)[:, :, HD:HD + 1],
                        1.0)
            else:
                dst = vt[:, (t - 1) * H * VS:t * H * VS]
                nc.vector.tensor_copy(
                    dst.rearrange("p (h v) -> p h v", h=H)[:, :, 0:HD],
                    ps[:].rearrange("p (h v) -> p h v", h=H))
                nc.vector.memset(
                    dst.rearrange("p (h v) -> p h v", h=H)[:, :, HD:HD + 1], 1.0)

        # round 0/1 operands first so attention starts as early as possible;
        # rounds 2/3's projections are emitted between rounds (see below) so
        # early attention work wins the scheduler's priority tie-breaks
        emit_qT(0)
        emit_kT(0)
        emit_qT(1)
        emit_kT(1)
        for t in range(7):
            emit_v(t)

        # ---- attention: 4 rounds of one head-group (4 heads) each ----
        # ctx psum per round: [128, 3*132] (qs 0-2) + [128, 132] (qs 3), so
        # all four query tiles fit two banks and the score pool can go deep.
        stg = const.tile([128, 4 * H * VS], f32)   # qs-major, head h at 33h
        ctxn = const.tile([128, 4 * SB], bf16)     # [q, D] normalized, per qs
        ctxT = const.tile([128, 4 * SB], bf16)
        STG = H * VS                               # 528 cols per qs block
        for hg in range(4):
            if hg == 2:
                emit_qT(2)
                emit_kT(2)
                emit_qT(3)
                emit_kT(3)
            # mem-key scores, 4 heads stacked in one psum
            ps_m = psum.tile([128, SB], f32, tag="proj")
            for hh in range(4):
                nc.tensor.matmul(
                    ps_m[32 * hh:32 * hh + MEM, :],
                    kT[32 * hh:32 * (hh + 1), KT * hg:KT * hg + MEM],
                    qT[32 * hh:32 * (hh + 1), SB * hg:SB * (hg + 1)],
                    start=True, stop=True, tile_position=(32 * hh, 32 * hh),
                )
            wm4 = work.tile([128, SB], bf16, tag="wm", bufs=2)
            nc.scalar.activation(wm4[:], ps_m[:], EXP)

            ctx_a = psum.tile([128, 3 * 4 * VS], f32, tag="ctx", bufs=2,
                              name=f"ctxa{hg}")
            ctx_b = psum.tile([128, 4 * VS], f32, tag="ctx", bufs=2,
                              name=f"ctxb{hg}")
            for hh in range(4):
                h = 4 * hg + hh
                kTh = kT[32 * hh:32 * (hh + 1), KT * hg:KT * (hg + 1)]
                qTh = qT[32 * hh:32 * (hh + 1), SB * hg:SB * (hg + 1)]

                wc = work.tile([128, WCW], bf16, tag="wc", bufs=4)
                for grp in CGROUPS:
                    gbase = CBASE[grp[0]]
                    ps_s = psum.tile([128, 512], f32, tag="sc", bufs=4)
                    gspan = 0
                    for c in grp:
                        qw0, qw1 = _chunk_qwin(c)
                        lc = CBASE[c] - gbase
                        nc.tensor.matmul(
                            ps_s[:, lc:lc + qw1 - qw0],
                            kTh[:, MEM + 128 * c:MEM + 128 * (c + 1)],
                            qTh[:, qw0:qw1], start=True, stop=True,
                            tile_position=(32 * hh, 0),
                        )
                        gspan = lc + qw1 - qw0
                    nc.scalar.activation(wc[:, gbase:gbase + gspan],
                                         ps_s[:, 0:gspan], EXP)
                nc.vector.tensor_mul(wc[:], wc[:], maskC[:])

                for qs in range(4):
                    if qs < 3:
                        out_ps = ctx_a[:, 4 * VS * qs + VS * hh:
                                       4 * VS * qs + VS * (hh + 1)]
                    else:
                        out_ps = ctx_b[:, VS * hh:VS * (hh + 1)]
                    nc.tensor.matmul(
                        out_ps,
                        wm4[32 * hh:32 * hh + MEM, 128 * qs:128 * (qs + 1)],
                        vt_mem4[32 * hh:32 * hh + MEM, VS * h:VS * (h + 1)],
                        start=True, stop=False, skip_group_check=True,
                        tile_position=(32 * hh, 0),
                    )
                    for ci, c in enumerate((qs, qs + 1, qs + 2)):
                        qw0, _ = _chunk_qwin(c)
                        nc.tensor.matmul(
                            out_ps,
                            wc[:, CBASE[c] + 128 * qs - qw0:
                                CBASE[c] + 128 * qs - qw0 + 128],
                            vt[:, c * H * VS + VS * h:c * H * VS + VS * (h + 1)],
                            start=False, stop=(ci == 2), skip_group_check=True,
                        )
            # evac this round's ctx into stg (head h at col 528*qs + 33*h)
            nc.scalar.copy(
                stg[:].rearrange("p (q n) -> p q n", q=4)[
                    :, 0:3, 4 * VS * hg:4 * VS * (hg + 1)],
                ctx_a[:].rearrange("p (q n) -> p q n", q=3))
            nc.scalar.copy(stg[:, 3 * STG + 4 * VS * hg:3 * STG + 4 * VS * (hg + 1)],
                           ctx_b[:])
            # normalize + transpose this round's heads (D-block dc == hg)
            for qs in range(4):
                blk = stg[:, STG * qs + 4 * VS * hg:
                          STG * qs + 4 * VS * (hg + 1)].rearrange(
                    "p (h v) -> p h v", h=4)
                rec = work.tile([128, 4], f32, tag="rec")
                nc.vector.reciprocal(rec[:].unsqueeze(2), blk[:, :, HD:HD + 1])
                nc.vector.tensor_mul(
                    ctxn[:, SB * qs + 128 * hg:SB * qs + 128 * (hg + 1)]
                    .rearrange("p (h v) -> p h v", h=4),
                    blk[:, :, 0:HD],
                    rec[:].unsqueeze(2).broadcast_to([128, 4, HD]))
                ps_t = psum.tile([128, 128], bf16, tag="ctx", bufs=2)
                nc.tensor.transpose(
                    ps_t[:],
                    ctxn[:, SB * qs + 128 * hg:SB * qs + 128 * (hg + 1)],
                    ident)
                nc.vector.tensor_copy(
                    ctxT[:, SB * hg + 128 * qs:SB * hg + 128 * (qs + 1)],
                    ps_t[:])

        # ---- output projection ----
        o_sb = const.tile([128, 4 * D], f32)
        for t in range(4):
            ps_o = psum.tile([128, D], f32, tag="proj")
            if has_bias:
                nc.tensor.matmul(ps_o[:], ones_row[:, 0:128], bo[:],
                                 start=True, stop=False)
            for dc in range(4):
                nc.tensor.matmul(
                    ps_o[:],
                    ctxT[:, SB * dc + 128 * t:SB * dc + 128 * (t + 1)],
                    wo[:, D * dc:D * (dc + 1)],
                    start=(dc == 0 and not has_bias), stop=(dc == 3),
                )
            nc.scalar.copy(o_sb[:, D * t:D * (t + 1)], ps_o[:])
            nc.sync.dma_start(out_d[128 * t:128 * (t + 1), :],
                              o_sb[:, D * t:D * (t + 1)])

    nc.compile()
    return nc


def _host_prep(inputs, has_bias):
    """Build the 8 per-core input maps."""
    q = np.asarray(inputs["query"], np.float32)
    k = np.asarray(inputs["key"], np.float32)
    v = np.asarray(inputs["value"], np.float32)
    mem = np.asarray(inputs["mem"], np.float32)[0]          # [16, 512]
    Wq = np.asarray(inputs["Wq"], np.float32) * SCALE
    bq = np.asarray(inputs["bq"], np.float32) * SCALE

    shared = {
        "wq": Wq.astype(BF16),
        "wk": np.asarray(inputs["Wk"], np.float32).astype(BF16),
        "wv": np.asarray(inputs["Wv"], np.float32).astype(BF16),
        "wo": np.asarray(inputs["Wo"], np.float32).astype(BF16),
    }
    if has_bias:
        shared["bq"] = bq.reshape(1, D).astype(BF16)
        for nm in ("bk", "bv", "bo"):
            shared[nm] = np.asarray(inputs[nm], np.float32).reshape(1, D).astype(BF16)

    memT = mem.T.astype(BF16)                                # [512, 16]
    in_maps = []
    for c in range(NCORES):
        b, blk = divmod(c, 4)
        s0 = blk * SB
        g0, g1 = s0 - HALO, s0 + SB + HALO
        lo, hi = max(0, g0), min(S, g1)
        xk_h = np.zeros((KC, D), np.float32)
        xv_h = np.zeros((KC, D), np.float32)
        xk_h[lo - g0:hi - g0] = k[b, lo:hi]
        xv_h[lo - g0:hi - g0] = v[b, lo:hi]
        # per-core mask template: 0/1 band per chunk, tightly packed per
        # CBASE, with out-of-sequence halo keys (whole partitions) zeroed
        cc = np.arange(KC) + g0
        kvalid = ((cc >= 0) & (cc < S)).astype(np.float32)
        maskC = np.zeros((128, WCW), np.float32)
        for ch in range(NKC):
            qw0, qw1 = _chunk_qwin(ch)
            p = np.arange(128)[:, None]
            n = np.arange(qw1 - qw0)[None, :]
            d = p + (128 * ch - qw0) - n
            band = ((d >= 0) & (d <= 2 * W2)).astype(np.float32)
            band *= kvalid[128 * ch:128 * (ch + 1)][:, None]
            maskC[:, CBASE[ch]:CBASE[ch] + qw1 - qw0] = band
        aux = np.zeros((128, WCW + 128), np.float32)
        aux[:, 0:WCW] = maskC
        aux[:, WCW:] = np.eye(128, dtype=np.float32)
        in_maps.append({
            "xq": np.ascontiguousarray(q[b, s0:s0 + SB].T).astype(BF16),
            "xk": np.concatenate([memT, xk_h.T.astype(BF16)], axis=1),
            "xv": np.concatenate([memT, xv_h.T.astype(BF16)], axis=1),
            "aux": aux.astype(BF16),
            **shared,
        })
    return in_maps


def kernel(**inputs):
    global LAST_RESULT
    from concourse.bass_utils import run_bass_kernel_spmd

    has_bias = any(
        np.any(np.asarray(inputs[n]) != 0) for n in ("bq", "bk", "bv", "bo"))
    key = ("nc", has_bias)
    if key not in _CACHE:
        _CACHE[key] = _build(has_bias)
    nc = _CACHE[key]

    in_maps = _host_prep(inputs, has_bias)
    res = run_bass_kernel_spmd(nc, in_maps, list(range(NCORES)))
    LAST_RESULT = res

    out = np.empty((B, S, D), np.float32)
    for c in range(NCORES):
        b, blk = divmod(c, 4)
        out[b, blk * SB:(blk + 1) * SB] = res.results[c]["out"]
    return out
